# revision 48
# baseline (speedup 1.0000x reference)
"""Trainium2 Bass kernel for char-CNN: 5-tap conv along word_length + max-pool.

Reference computation (per (batch, sentence) word, shapes B=64 S=256 W=20 E=128):
    y[w, e] = sum_{kh=0..4} x[w + kh - 2, e] * conv_w[kh]     (zero padded)
    out[e]  = max_w y[w, e] + conv_b

Strategy:
  - Data-parallel over 8 NeuronCores: 8 batches (2048 words) per core.
  - Host pre-arranges each core's shard to z[(j w)=120, group=342, e=128]
    (groups of J=6 words, last group zero-padded) so every DMA descriptor
    is a multi-KiB contiguous run per partition — full HBM bandwidth.
  - The conv is a banded 20x20 matrix applied per word, done on TensorE:
    stationary lhsT = x6 [K=120 (6 words x 20 w_in), M=128 (e)], moving
    rhs = block-diagonal A [120, 120] -> PSUM [128 (e), 120 (6w x 20 w_out)].
    fp16 operands (1 cycle/row on PE; fp32 would be 4).
  - Max over w_out is a free-dim reduce on VectorE straight out of PSUM:
    [128, (groups, 20)] -> [128, groups*6] into a persistent [128, NW]
    maxima tile; one DMA out at the end (host transposes back).
  - Input DMAs are spread across the SP-HWDGE / ACT-HWDGE / SWDGE rings so
    the 16 SDMA engines always have in-flight work (one FIFO ring alone
    leaves completion-latency bubbles).  The SWDGE (gpsimd) ring casts
    f32 -> f16 in the DMA datapath; HWDGE rings land f32 and ScalarE casts.
"""

from contextlib import ExitStack

import numpy as np

import concourse.bass as bass
import concourse.mybir as mybir
import concourse.tile as tile
from concourse import bacc

W = 20  # word length
E = 128  # embedding dim
KH = 5  # conv taps
PAD = 2
J = 6  # words per matmul group (6 * 20 = 120 <= 128 partitions)
KP = J * W  # contraction size / partitions used (120)
CG = 16  # groups per compute sub-chunk (4 PSUM banks)
NCORES = 8
BANK = 512  # PSUM bank size in f32 elements


def build_conv_matrix(conv_w: np.ndarray) -> np.ndarray:
    """Block-diagonal [KP, KP] matrix: A[j*W+wi, j*W+wo] = conv_w[wi-wo+2]."""
    wv = np.asarray(conv_w, np.float32).reshape(-1)
    assert wv.shape == (KH,)
    blk = np.zeros((W, W), np.float32)
    for wo in range(W):
        for kh in range(KH):
            wi = wo + kh - PAD
            if 0 <= wi < W:
                blk[wi, wo] = wv[kh]
    a = np.zeros((KP, KP), np.float32)
    for j in range(J):
        a[j * W : (j + 1) * W, j * W : (j + 1) * W] = blk
    return a.astype(np.float16)


def pack_input(x_core: np.ndarray, ng: int) -> np.ndarray:
    """[nw, W, E] f32 -> [KP, ng, E] f16 partition-major, zero-padded to
    ng*J words. The fp16 cast is the same one the kernel's compute path
    uses (TensorE consumes fp16); doing it host-side halves HBM traffic."""
    nw = x_core.shape[0]
    xp = np.zeros((ng * J, W, E), np.float16)
    xp[:nw] = x_core.astype(np.float16)
    # (g j) w e -> (j w) g e
    return np.ascontiguousarray(
        xp.reshape(ng, J, W, E).transpose(1, 2, 0, 3).reshape(KP, ng, E)
    )


def chunk_plan(ng: int, big: int = 64) -> list[int]:
    """Descending chunk sizes: big early (fewer ring bubbles while the
    stream is deep), small at the end (short pipeline tail)."""
    sizes = []
    rem = ng
    for sz, keep in ((64, 96), (32, 48), (16, 24), (8, 8)):
        if sz > big:
            continue
        while rem >= max(sz, keep):
            sizes.append(sz)
            rem -= sz
    if rem:
        sizes.append(rem)
    return sizes


def build_nc(
    nw: int,
    dma_rings: tuple[str, ...] = ("gpsimd",),
    bufs: int = 24,
    first_ring: str | None = None,
    big_chunk: int = 16,
    cg: int = 16,
    psum_bufs: int = 2,
    d_every: int = 12,
) -> bass.Bass:
    """Build the per-core Bass graph. nw = real words per core.

    dma_rings: which descriptor rings carry the input stream, round-robin
    per chunk. 'gpsimd' (SWDGE) casts f32->f16 during the DMA; HWDGE rings
    ('sync'/'scalar') land f32 and ScalarE casts to f16.
    """
    f32 = mybir.dt.float32
    f16 = mybir.dt.float16
    ng = (nw + J - 1) // J  # padded group count
    nwp = ng * J  # padded word count

    nc = bacc.Bacc()
    z_ext = nc.declare_dram_parameter("z", [KP, ng, E], f16, isOutput=False)
    a_ext = nc.declare_dram_parameter("a", [KP, KP], f16, isOutput=False)
    out_ext = nc.declare_dram_parameter("out", [E, nw], f32, isOutput=True)

    engines = {
        "sync": nc.sync,
        "scalar": nc.scalar,
        "gpsimd": nc.gpsimd,
    }

    with ExitStack() as ctx:
        tc = ctx.enter_context(tile.TileContext(nc))
        const = ctx.enter_context(tc.tile_pool(name="const", bufs=1))
        hpool = ctx.enter_context(tc.tile_pool(name="xh", bufs=bufs))
        opool = ctx.enter_context(tc.tile_pool(name="o", bufs=1))
        spool = ctx.enter_context(tc.tile_pool(name="ys", bufs=8))
        t1pool = ctx.enter_context(tc.tile_pool(name="t1", bufs=8))
        pspool = ctx.enter_context(
            tc.tile_pool(name="ps", bufs=psum_bufs, space="PSUM")
        )
        ps_banks = (cg + 3) // 4  # PSUM banks per compute sub-chunk

        a_t = const.tile([KP, KP], f16)
        nc.sync.dma_start(out=a_t[:, :], in_=a_ext[:, :])
        maxt = opool.tile([E, nwp], f32)

        HW = W // 2  # 10

        def do_matmuls(xh, coff, sn):
            ps = pspool.tile([E, ps_banks * BANK], f32, tag="ps")
            for g in range(sn):
                col = (g // 4) * BANK + (g % 4) * KP
                nc.tensor.matmul(
                    ps[:, col : col + KP],
                    lhsT=xh[:, coff + g * E : coff + (g + 1) * E],
                    rhs=a_t[:, :],
                    start=True,
                    stop=True,
                )
            return ps

        def psum_view(ps, sn):
            """[E, nbank, c, W] view of sn (multiple of 4) groups."""
            nbank = sn // 4
            return (
                ps[:, 0 : nbank * BANK]
                .rearrange("p (k x) -> p k x", k=nbank)[:, :, 0 : 4 * J * W]
                .rearrange("p k (c w) -> p k c w", w=W)
            )

        def stage12_act(xh, coff, sg0, sn):
            """ACT parks the whole 20-block in SBUF as f16 (its own PSUM
            port), DVE folds 20 -> 10 in f16 2x mode. sn % 4 == 0."""
            blocks = sn * J
            ps = do_matmuls(xh, coff, sn)
            pv = psum_view(ps, sn)
            s = spool.tile([E, cg * J * W], f16, tag="ys")
            sv = s[:, 0 : blocks * W].rearrange(
                "p (k c w) -> p k c w", k=sn // 4, w=W
            )
            nc.scalar.copy(sv, pv)
            t1 = t1pool.tile([E, cg * J * HW], f16, tag="t1")
            sb = s[:, 0 : blocks * W].rearrange("p (c w) -> p c w", w=W)
            t1v = t1[:, 0 : blocks * HW].rearrange("p (c w) -> p c w", w=HW)
            nc.vector.tensor_max(t1v, sb[:, :, 0:HW], sb[:, :, HW:W])
            return ("a", t1, sg0, sn)

        def stage12_direct(xh, coff, sg0, sn):
            ps = do_matmuls(xh, coff, sn)
            return ("d", ps, sg0, sn)

        def stage3(kind, t, sg0, sn):
            blocks = sn * J
            if kind == "a":
                t1v = t[:, 0 : blocks * HW].rearrange("p (c w) -> p c w", w=HW)
                nc.vector.reduce_max(
                    maxt[:, sg0 * J : sg0 * J + blocks],
                    t1v,
                    axis=mybir.AxisListType.X,
                )
                return
            # direct: reduce 20 straight out of PSUM
            if sn % 4 == 0:
                pv = psum_view(t, sn)
                out_v = maxt[:, sg0 * J : sg0 * J + blocks].rearrange(
                    "p (k c) -> p k c", k=sn // 4
                )
                nc.vector.reduce_max(out_v, pv, axis=mybir.AxisListType.X)
                return
            wcur = sg0 * J
            for b in range((sn + 3) // 4):
                gb = min(4, sn - 4 * b)
                cb = gb * J
                pv = t[:, BANK * b : BANK * b + cb * W].rearrange(
                    "p (c w) -> p c w", w=W
                )
                nc.vector.reduce_max(
                    maxt[:, wcur : wcur + cb],
                    pv,
                    axis=mybir.AxisListType.X,
                )
                wcur += cb

        g0 = 0
        if first_ring is not None:
            sizes = [16] + chunk_plan(ng - 16, big_chunk)
            rings = [first_ring] + [
                dma_rings[i % len(dma_rings)] for i in range(len(sizes) - 1)
            ]
        else:
            if ng > 32:
                # small warm-up chunks so compute starts sooner
                sizes = [4, 4, 8] + chunk_plan(ng - 16, big_chunk)
            else:
                sizes = chunk_plan(ng, big_chunk)
            rings = [dma_rings[i % len(dma_rings)] for i in range(len(sizes))]
        max_gn = max(sizes)

        # Phase A: the whole input stream is issued up front (bufs covers
        # every chunk) so no compute op can head-of-line-block a DMA
        # trigger on the gpsimd FIFO.
        subs = []
        for ring, gn in enumerate(sizes):
            eng_name = rings[ring]
            src = z_ext[:, g0 : g0 + gn, :].rearrange("p g e -> p (g e)")
            xh = hpool.tile([KP, max_gn * E], f16, tag="xh")
            engines[eng_name].dma_start(out=xh[:, 0 : gn * E], in_=src)
            for s0 in range(0, gn, cg):
                sn = min(cg, gn - s0)
                subs.append((xh, s0 * E, g0 + s0, sn))
            g0 += gn

        # Phase B: compute pipeline. stage3 follows its stage12 directly:
        # its dependency is the immediately preceding same-engine op (the
        # DVE tensor_max for ACT-path subs, the PE matmuls for direct).
        w_flushed = 0

        def flush_out(upto_words):
            nonlocal w_flushed
            hi = min(upto_words, nw)
            if hi - w_flushed >= 192 or (hi >= nw and hi > w_flushed):
                nc.sync.dma_start(
                    out=out_ext[:, w_flushed:hi], in_=maxt[:, w_flushed:hi]
                )
                w_flushed = hi

        for idx, sub in enumerate(subs):
            _, _, _, sn = sub
            if sn % 4 == 0 and (d_every == 0 or idx % d_every != d_every - 1):
                kind, t, sg0, sn = stage12_act(*sub)
            else:
                kind, t, sg0, sn = stage12_direct(*sub)
            stage3(kind, t, sg0, sn)
            flush_out(sg0 * J + sn * J)
    nc.finalize()
    return nc


def kernel(embedded_char, conv_w, conv_b):
    from concourse.bass_utils import run_bass_kernel_spmd

    x = np.asarray(embedded_char, np.float32)
    b_val = float(np.asarray(conv_b, np.float32).reshape(-1)[0])
    B, S, Wl, El = x.shape
    assert (Wl, El) == (W, E)
    bs = B // NCORES
    nw = bs * S
    ng = (nw + J - 1) // J
    a16 = build_conv_matrix(conv_w)

    nc = build_nc(nw)
    in_maps = [
        {
            "z": pack_input(x[i * bs : (i + 1) * bs].reshape(nw, Wl, El), ng),
            "a": a16,
        }
        for i in range(NCORES)
    ]
    res = run_bass_kernel_spmd(nc, in_maps, core_ids=list(range(NCORES)))
    full = np.concatenate(
        [r["out"].T.reshape(bs, S, El) for r in res.results], axis=0
    )
    if b_val != 0.0:
        full = full + b_val
    return np.ascontiguousarray(full.astype(np.float32))


# revision 49
# speedup vs baseline: 1.0312x; 1.0312x over previous
"""Trainium2 Bass kernel for char-CNN: 5-tap conv along word_length + max-pool.

Reference computation (per (batch, sentence) word, shapes B=64 S=256 W=20 E=128):
    y[w, e] = sum_{kh=0..4} x[w + kh - 2, e] * conv_w[kh]     (zero padded)
    out[e]  = max_w y[w, e] + conv_b

Strategy:
  - Data-parallel over 8 NeuronCores: 8 batches (2048 words) per core.
  - Host pre-arranges each core's shard to z[(j w)=120, group=342, e=128]
    (groups of J=6 words, last group zero-padded) so every DMA descriptor
    is a multi-KiB contiguous run per partition — full HBM bandwidth.
  - The conv is a banded 20x20 matrix applied per word, done on TensorE:
    stationary lhsT = x6 [K=120 (6 words x 20 w_in), M=128 (e)], moving
    rhs = block-diagonal A [120, 120] -> PSUM [128 (e), 120 (6w x 20 w_out)].
    fp16 operands (1 cycle/row on PE; fp32 would be 4).
  - Max over w_out is a free-dim reduce on VectorE straight out of PSUM:
    [128, (groups, 20)] -> [128, groups*6] into a persistent [128, NW]
    maxima tile; one DMA out at the end (host transposes back).
  - Input DMAs are spread across the SP-HWDGE / ACT-HWDGE / SWDGE rings so
    the 16 SDMA engines always have in-flight work (one FIFO ring alone
    leaves completion-latency bubbles).  The SWDGE (gpsimd) ring casts
    f32 -> f16 in the DMA datapath; HWDGE rings land f32 and ScalarE casts.
"""

from contextlib import ExitStack

import numpy as np

import concourse.bass as bass
import concourse.mybir as mybir
import concourse.tile as tile
from concourse import bacc

W = 20  # word length
E = 128  # embedding dim
KH = 5  # conv taps
PAD = 2
J = 6  # words per matmul group (6 * 20 = 120 <= 128 partitions)
KP = J * W  # contraction size / partitions used (120)
CG = 16  # groups per compute sub-chunk (4 PSUM banks)
NCORES = 8
BANK = 512  # PSUM bank size in f32 elements


def build_conv_matrix(conv_w: np.ndarray) -> np.ndarray:
    """Block-diagonal [KP, KP] matrix: A[j*W+wi, j*W+wo] = conv_w[wi-wo+2]."""
    wv = np.asarray(conv_w, np.float32).reshape(-1)
    assert wv.shape == (KH,)
    blk = np.zeros((W, W), np.float32)
    for wo in range(W):
        for kh in range(KH):
            wi = wo + kh - PAD
            if 0 <= wi < W:
                blk[wi, wo] = wv[kh]
    a = np.zeros((KP, KP), np.float32)
    for j in range(J):
        a[j * W : (j + 1) * W, j * W : (j + 1) * W] = blk
    return a.astype(np.float16)


def pack_input(x_core: np.ndarray, ng: int) -> np.ndarray:
    """[nw, W, E] f32 -> [KP, ng, E] f16 partition-major, zero-padded to
    ng*J words. The fp16 cast is the same one the kernel's compute path
    uses (TensorE consumes fp16); doing it host-side halves HBM traffic."""
    nw = x_core.shape[0]
    xp = np.zeros((ng * J, W, E), np.float16)
    xp[:nw] = x_core.astype(np.float16)
    # (g j) w e -> (j w) g e
    return np.ascontiguousarray(
        xp.reshape(ng, J, W, E).transpose(1, 2, 0, 3).reshape(KP, ng, E)
    )


def chunk_plan(ng: int, big: int = 64) -> list[int]:
    """Descending chunk sizes: big early (fewer ring bubbles while the
    stream is deep), small at the end (short pipeline tail)."""
    sizes = []
    rem = ng
    for sz, keep in ((64, 96), (32, 48), (16, 24), (8, 8)):
        if sz > big:
            continue
        while rem >= max(sz, keep):
            sizes.append(sz)
            rem -= sz
    if rem:
        sizes.append(rem)
    return sizes


def build_nc(
    nw: int,
    dma_rings: tuple[str, ...] = ("gpsimd",),
    bufs: int = 24,
    first_ring: str | None = None,
    big_chunk: int = 16,
    cg: int = 16,
    psum_bufs: int = 2,
    d_every: int = 12,
) -> bass.Bass:
    """Build the per-core Bass graph. nw = real words per core.

    dma_rings: which descriptor rings carry the input stream, round-robin
    per chunk. 'gpsimd' (SWDGE) casts f32->f16 during the DMA; HWDGE rings
    ('sync'/'scalar') land f32 and ScalarE casts to f16.
    """
    f32 = mybir.dt.float32
    f16 = mybir.dt.float16
    ng = (nw + J - 1) // J  # padded group count
    nwp = ng * J  # padded word count

    nc = bacc.Bacc()
    z_ext = nc.declare_dram_parameter("z", [KP, ng, E], f16, isOutput=False)
    a_ext = nc.declare_dram_parameter("a", [KP, KP], f16, isOutput=False)
    out_ext = nc.declare_dram_parameter("out", [E, nw], f32, isOutput=True)

    engines = {
        "sync": nc.sync,
        "scalar": nc.scalar,
        "gpsimd": nc.gpsimd,
    }

    with ExitStack() as ctx:
        tc = ctx.enter_context(tile.TileContext(nc))
        const = ctx.enter_context(tc.tile_pool(name="const", bufs=1))
        hpool = ctx.enter_context(tc.tile_pool(name="xh", bufs=bufs))
        opool = ctx.enter_context(tc.tile_pool(name="o", bufs=1))
        spool = ctx.enter_context(tc.tile_pool(name="ys", bufs=8))
        t1pool = ctx.enter_context(tc.tile_pool(name="t1", bufs=8))
        pspool = ctx.enter_context(
            tc.tile_pool(name="ps", bufs=psum_bufs, space="PSUM")
        )
        ps_banks = (cg + 3) // 4  # PSUM banks per compute sub-chunk

        a_t = const.tile([KP, KP], f16)
        nc.sync.dma_start(out=a_t[:, :], in_=a_ext[:, :])
        maxt = opool.tile([E, nwp], f32)

        HW = W // 2  # 10

        def do_matmuls(xh, coff, sn):
            ps = pspool.tile([E, ps_banks * BANK], f32, tag="ps")
            for g in range(sn):
                col = (g // 4) * BANK + (g % 4) * KP
                nc.tensor.matmul(
                    ps[:, col : col + KP],
                    lhsT=xh[:, coff + g * E : coff + (g + 1) * E],
                    rhs=a_t[:, :],
                    start=True,
                    stop=True,
                )
            return ps

        def psum_view(ps, sn):
            """[E, nbank, c, W] view of sn (multiple of 4) groups."""
            nbank = sn // 4
            return (
                ps[:, 0 : nbank * BANK]
                .rearrange("p (k x) -> p k x", k=nbank)[:, :, 0 : 4 * J * W]
                .rearrange("p k (c w) -> p k c w", w=W)
            )

        def stage12_act(xh, coff, sg0, sn):
            """ACT parks the whole 20-block in SBUF as f16 (its own PSUM
            port), DVE folds 20 -> 10 in f16 2x mode. sn % 4 == 0."""
            blocks = sn * J
            ps = do_matmuls(xh, coff, sn)
            pv = psum_view(ps, sn)
            s = spool.tile([E, cg * J * W], f16, tag="ys")
            sv = s[:, 0 : blocks * W].rearrange(
                "p (k c w) -> p k c w", k=sn // 4, w=W
            )
            nc.scalar.copy(sv, pv)
            t1 = t1pool.tile([E, cg * J * HW], f16, tag="t1")
            sb = s[:, 0 : blocks * W].rearrange("p (c w) -> p c w", w=W)
            t1v = t1[:, 0 : blocks * HW].rearrange("p (c w) -> p c w", w=HW)
            nc.vector.tensor_max(t1v, sb[:, :, 0:HW], sb[:, :, HW:W])
            return ("a", t1, sg0, sn)

        def stage12_direct(xh, coff, sg0, sn):
            ps = do_matmuls(xh, coff, sn)
            return ("d", ps, sg0, sn)

        def stage3(kind, t, sg0, sn):
            blocks = sn * J
            if kind == "a":
                t1v = t[:, 0 : blocks * HW].rearrange("p (c w) -> p c w", w=HW)
                nc.vector.reduce_max(
                    maxt[:, sg0 * J : sg0 * J + blocks],
                    t1v,
                    axis=mybir.AxisListType.X,
                )
                return
            # direct: reduce 20 straight out of PSUM
            if sn % 4 == 0:
                pv = psum_view(t, sn)
                out_v = maxt[:, sg0 * J : sg0 * J + blocks].rearrange(
                    "p (k c) -> p k c", k=sn // 4
                )
                nc.vector.reduce_max(out_v, pv, axis=mybir.AxisListType.X)
                return
            wcur = sg0 * J
            for b in range((sn + 3) // 4):
                gb = min(4, sn - 4 * b)
                cb = gb * J
                pv = t[:, BANK * b : BANK * b + cb * W].rearrange(
                    "p (c w) -> p c w", w=W
                )
                nc.vector.reduce_max(
                    maxt[:, wcur : wcur + cb],
                    pv,
                    axis=mybir.AxisListType.X,
                )
                wcur += cb

        g0 = 0
        if first_ring is not None:
            sizes = [16] + chunk_plan(ng - 16, big_chunk)
            rings = [first_ring] + [
                dma_rings[i % len(dma_rings)] for i in range(len(sizes) - 1)
            ]
        else:
            if ng > 32:
                # two small warm-up chunks so compute starts sooner
                sizes = [8, 8] + chunk_plan(ng - 16, big_chunk)
            else:
                sizes = chunk_plan(ng, big_chunk)
            rings = [dma_rings[i % len(dma_rings)] for i in range(len(sizes))]
        max_gn = max(sizes)

        # Phase A: the whole input stream is issued up front (bufs covers
        # every chunk) so no compute op can head-of-line-block a DMA
        # trigger on the gpsimd FIFO.
        subs = []
        for ring, gn in enumerate(sizes):
            eng_name = rings[ring]
            src = z_ext[:, g0 : g0 + gn, :].rearrange("p g e -> p (g e)")
            xh = hpool.tile([KP, max_gn * E], f16, tag="xh")
            engines[eng_name].dma_start(out=xh[:, 0 : gn * E], in_=src)
            for s0 in range(0, gn, cg):
                sn = min(cg, gn - s0)
                subs.append((xh, s0 * E, g0 + s0, sn))
            g0 += gn

        # Phase B: compute pipeline. stage3 follows its stage12 directly:
        # its dependency is the immediately preceding same-engine op (the
        # DVE tensor_max for ACT-path subs, the PE matmuls for direct).
        w_flushed = 0

        def flush_out(upto_words):
            nonlocal w_flushed
            hi = min(upto_words, nw)
            if hi - w_flushed >= 192 or (hi >= nw and hi > w_flushed):
                nc.sync.dma_start(
                    out=out_ext[:, w_flushed:hi], in_=maxt[:, w_flushed:hi]
                )
                w_flushed = hi

        for idx, sub in enumerate(subs):
            _, _, _, sn = sub
            if sn % 4 == 0 and (d_every == 0 or idx % d_every != d_every - 1):
                kind, t, sg0, sn = stage12_act(*sub)
            else:
                kind, t, sg0, sn = stage12_direct(*sub)
            stage3(kind, t, sg0, sn)
            flush_out(sg0 * J + sn * J)
    nc.finalize()
    return nc


def kernel(embedded_char, conv_w, conv_b):
    from concourse.bass_utils import run_bass_kernel_spmd

    x = np.asarray(embedded_char, np.float32)
    b_val = float(np.asarray(conv_b, np.float32).reshape(-1)[0])
    B, S, Wl, El = x.shape
    assert (Wl, El) == (W, E)
    bs = B // NCORES
    nw = bs * S
    ng = (nw + J - 1) // J
    a16 = build_conv_matrix(conv_w)

    nc = build_nc(nw)
    in_maps = [
        {
            "z": pack_input(x[i * bs : (i + 1) * bs].reshape(nw, Wl, El), ng),
            "a": a16,
        }
        for i in range(NCORES)
    ]
    res = run_bass_kernel_spmd(nc, in_maps, core_ids=list(range(NCORES)))
    full = np.concatenate(
        [r["out"].T.reshape(bs, S, El) for r in res.results], axis=0
    )
    if b_val != 0.0:
        full = full + b_val
    return np.ascontiguousarray(full.astype(np.float32))
